# revision 32
# baseline (speedup 1.0000x reference)
"""Masked multi-head attention (B=4, S=2048, E=768, H=12) on 8 TRN2 NeuronCores.

Sharding: core c handles batch b=c//2 and query-half qh=c%2 (1024 queries).
Each core runs the full pipeline independently (pure data parallel):
  - project Q (its q-half) and K/V (full seq of its batch, duplicated
    across the 2 cores that share a batch),
  - masked softmax(QK^T/sqrt(E)) computed in transposed layout S^T[k, q]
    (mask folded in post-exp as a 0/1 bf16 multiply; row sums obtained by
    augmenting V with a block of ones columns so the AV matmul emits them),
  - attention output @ Wfc.

Everything TensorEngine-facing is bf16; accumulation fp32 in PSUM.
"""

import math

import numpy as np

B, S, E, H, D = 4, 2048, 768, 12, 64
QH = S // 2               # queries per core
G = H // 2                # head pairs
EC = E // 128             # embed-dim chunks
KT = S // 128             # key tiles
VB = 192 * 6              # v-buffer cols/tile: 6 x [V_2g(64)|ones(64)|V_2g+1(64)]
SCALE = 1.0 / math.sqrt(E)
N_CORES = 8

_CACHE = {}


def _build(debug_taps=False, reps=1):
    import concourse.bass as bass
    import concourse.mybir as mybir
    import concourse.tile as tile
    from concourse import bacc

    f32 = mybir.dt.float32
    bf16 = mybir.dt.bfloat16
    i32 = mybir.dt.int32

    nc = bacc.Bacc("TRN2", target_bir_lowering=False, debug=False,
                   enable_asserts=False, num_devices=N_CORES)

    Qx = nc.dram_tensor("Q", [QH, E], f32, kind="ExternalInput").ap()
    Kx = nc.dram_tensor("K", [S, E], f32, kind="ExternalInput").ap()
    Vx = nc.dram_tensor("V", [S, E], f32, kind="ExternalInput").ap()
    Mx = nc.dram_tensor("mask", [QH, S], i32, kind="ExternalInput").ap()
    Wqx = nc.dram_tensor("Wq", [E, E], f32, kind="ExternalInput").ap()
    Wkx = nc.dram_tensor("Wk", [E, E], f32, kind="ExternalInput").ap()
    Wvx = nc.dram_tensor("Wv", [E, E], f32, kind="ExternalInput").ap()
    Wfcx = nc.dram_tensor("Wfc", [E, E], f32, kind="ExternalInput").ap()
    Ox = nc.dram_tensor("out", [QH, E], f32, kind="ExternalOutput").ap()

    qbf = nc.dram_tensor("qbf", [QH, E], bf16).ap()
    kbf = nc.dram_tensor("kbf", [S, E], bf16).ap()
    vbf = nc.dram_tensor("vbf", [S, E], bf16).ap()
    mbf = nc.dram_tensor("mbf", [QH, S], bf16).ap()

    dbg = {}
    if debug_taps:
        dbg["qtp"] = nc.dram_tensor("dbg_qtp", [128, EC, QH], f32, kind="ExternalOutput").ap()
        dbg["ktp"] = nc.dram_tensor("dbg_ktp", [128, EC, S], f32, kind="ExternalOutput").ap()
        dbg["vb"] = nc.dram_tensor("dbg_vb", [128, KT, VB], f32, kind="ExternalOutput").ap()
        dbg["attnT"] = nc.dram_tensor("dbg_attnT", [128, EC, QH], f32, kind="ExternalOutput").ap()
        dbg["mt"] = nc.dram_tensor("dbg_mt", [128, KT, 512], f32, kind="ExternalOutput").ap()
        dbg["st"] = nc.dram_tensor("dbg_st", [128, 4, 512], f32, kind="ExternalOutput").ap()
        dbg["po"] = nc.dram_tensor("dbg_po", [128, 512], f32, kind="ExternalOutput").ap()
        dbg["rza"] = nc.dram_tensor("dbg_rza", [128, 512], f32, kind="ExternalOutput").ap()

    ext = dict(Qx=Qx, Kx=Kx, Vx=Vx, Mx=Mx, Wqx=Wqx, Wkx=Wkx, Wvx=Wvx,
               Wfcx=Wfcx, Ox=Ox, qbf=qbf, kbf=kbf, vbf=vbf, mbf=mbf)

    with tile.TileContext(nc) as tc:
        with (
            tc.tile_pool(name="persist", bufs=1) as persist,
            tc.tile_pool(name="inT", bufs=2) as inT,
            tc.tile_pool(name="mtp", bufs=2) as mtp,
            tc.tile_pool(name="stp", bufs=4) as stp,
            tc.tile_pool(name="rzp", bufs=2) as rzp,
            tc.tile_pool(name="osbp", bufs=2) as osbp,
            tc.tile_pool(name="pep", bufs=3, space="PSUM") as pep,
            tc.tile_pool(name="pop", bufs=2, space="PSUM") as pop,
        ):
            pools = dict(persist=persist, inT=inT, mtp=mtp,
                         stp=stp, rzp=rzp, osbp=osbp, pep=pep, pop=pop)
            if reps == 1:
                _emit(nc, bass, mybir, pools, ext, dbg, debug_taps)
            else:
                with tc.For_i(0, reps, 1):
                    _emit(nc, bass, mybir, pools, ext, dbg, debug_taps)

    nc.compile()
    return nc


def _emit(nc, bass, mybir, pools, ext, dbg, debug_taps):
    f32 = mybir.dt.float32
    bf16 = mybir.dt.bfloat16
    i32 = mybir.dt.int32
    Exp = mybir.ActivationFunctionType.Exp
    Copy = mybir.ActivationFunctionType.Copy

    persist, inT = pools["persist"], pools["inT"]
    mtp, stp, rzp, osbp = pools["mtp"], pools["stp"], pools["rzp"], pools["osbp"]
    pep, pop = pools["pep"], pools["pop"]
    Qx, Kx, Vx, Mx = ext["Qx"], ext["Kx"], ext["Vx"], ext["Mx"]
    Wqx, Wkx, Wvx, Wfcx, Ox = ext["Wqx"], ext["Wkx"], ext["Wvx"], ext["Wfcx"], ext["Ox"]
    qbf, kbf, vbf, mbf = ext["qbf"], ext["kbf"], ext["vbf"], ext["mbf"]

    wq_sb = persist.tile([128, EC, E], bf16)
    wk_sb = persist.tile([128, EC, E], bf16)
    wv_sb = persist.tile([128, EC, E], bf16)
    wfc_sb = persist.tile([128, EC, E], bf16)
    qtp = persist.tile([128, EC, QH], bf16)     # projected Q^T
    ktp = persist.tile([128, EC, S], bf16)      # projected K^T
    vb = persist.tile([128, KT, VB], bf16)      # projected V (+ones)
    attnT = persist.tile([128, EC, QH], bf16)   # attn output^T

    # ---- weight loads: f32 via HWDGE + cast on the (prologue-idle) ACT ----
    def load_weight(w_sb, w_x, name):
        w_v = w_x.rearrange("(c p) e -> c p e", p=128)
        for c in range(EC):
            wst = osbp.tile([128, E], f32, tag="osb", name=f"wst_{name}{c}")
            nc.sync.dma_start(out=wst, in_=w_v[c])
            nc.scalar.activation(out=w_sb[:, c, :], in_=wst, func=Copy)

    # ---- SWDGE cast-DMA queue, in critical-path order ----
    nc.gpsimd.dma_start(out=qbf[:], in_=Qx[:])
    load_weight(wq_sb, Wqx, "q")
    nc.gpsimd.dma_start(out=kbf[0:QH], in_=Kx[0:QH])
    load_weight(wk_sb, Wkx, "k")
    nc.gpsimd.dma_start(out=vbf[0:QH], in_=Vx[0:QH])
    nc.gpsimd.dma_start(out=kbf[QH:S], in_=Kx[QH:S])
    nc.gpsimd.dma_start(out=vbf[QH:S], in_=Vx[QH:S])
    load_weight(wv_sb, Wvx, "v")
    load_weight(wfc_sb, Wfcx, "fc")

    # ---- mask int32 -> bf16 off the congested SWDGE queue: load on the
    # second HWDGE ring (ACT-issued), cast on the prologue-idle DVE ----
    for x in range(QH // 128):
        for y in range(2):
            mi = rzp.tile([128, QH], i32, tag="rz", name=f"mi{x}_{y}")
            nc.scalar.dma_start(
                out=mi, in_=Mx[x * 128:(x + 1) * 128, y * QH:(y + 1) * QH])
            mb = rzp.tile([128, QH], bf16, tag="rz", name=f"mb{x}_{y}")
            nc.vector.tensor_copy(out=mb, in_=mi)
            nc.scalar.dma_start(
                out=mbf[x * 128:(x + 1) * 128, y * QH:(y + 1) * QH], in_=mb)

    # ones blocks of the V buffer (shared within each head pair)
    for g in range(G):
        nc.vector.memset(vb[:, :, 192 * g + 64:192 * g + 128], 1.0)

    # ---- Q projection:  qtp[:, g, s] = (Q @ Wq)^T ----
    qbf_v = qbf.rearrange("s (c p) -> c s p", p=128)
    qtin = inT.tile([128, EC, QH], bf16, tag="int")
    for c in range(EC):
        nc.sync.dma_start(out=qtin[:, c, :], in_=qbf_v[c], transpose=True)
    for g in range(EC):
        for j in range(QH // 512):
            ps_w = pep.tile([128, 2, 512], f32, tag="pe", name="psq")
            ps = ps_w[:, 0, :]
            for fc in range(EC):
                nc.tensor.matmul(
                    out=ps, lhsT=wq_sb[:, fc, g * 128:(g + 1) * 128],
                    rhs=qtin[:, fc, j * 512:(j + 1) * 512],
                    start=(fc == 0), stop=(fc == EC - 1))
            nc.vector.tensor_copy(out=qtp[:, g, j * 512:(j + 1) * 512], in_=ps)

    # ---- K projection halves ----
    kbf_v = kbf.rearrange("(h s) (c p) -> h c s p", s=QH, p=128)

    def k_proj_half(half):
        ktin = inT.tile([128, EC, QH], bf16, tag="int", name=f"ktin{half}")
        for c in range(EC):
            nc.sync.dma_start(out=ktin[:, c, :], in_=kbf_v[half, c],
                              transpose=True)
        for g in range(EC):
            for j in range(2):
                ps_w = pep.tile([128, 2, 512], f32, tag="pe", name="psk")
                ps = ps_w[:, 0, :]
                for fc in range(EC):
                    nc.tensor.matmul(
                        out=ps, lhsT=wk_sb[:, fc, g * 128:(g + 1) * 128],
                        rhs=ktin[:, fc, j * 512:(j + 1) * 512],
                        start=(fc == 0), stop=(fc == EC - 1))
                nc.vector.tensor_copy(
                    out=ktp[:, g, half * 1024 + j * 512:half * 1024 + (j + 1) * 512],
                    in_=ps)

    # ---- V projection halves (into vb group columns) ----
    vbf_v = vbf.rearrange("(h s) (c p) -> h c s p", s=QH, p=128)

    def v_proj_half(half):
        vtin = inT.tile([128, EC, QH], bf16, tag="int", name=f"vtin{half}")
        for c in range(EC):
            nc.sync.dma_start(out=vtin[:, c, :], in_=vbf_v[half, c],
                              transpose=True)
        for kt8 in range(8):
            kt = half * 8 + kt8
            for eh in range(2):
                ps_w = pep.tile([128, 2, 512], f32, tag="pe", name="psv")
                ps = ps_w[:, 0, 0:384]
                for fc in range(EC):
                    nc.tensor.matmul(
                        out=ps, lhsT=vtin[:, fc, kt8 * 128:(kt8 + 1) * 128],
                        rhs=wv_sb[:, fc, eh * 384:(eh + 1) * 384],
                        start=(fc == 0), stop=(fc == EC - 1))
                # psum [128, 384] covers heads 6*eh..6*eh+5; dest columns
                # 192*pair + 128*(h%2) + j
                v0 = vb[:, kt, 576 * eh:576 * eh + 576]
                dst = bass.AP(tensor=v0.tensor, offset=v0.offset,
                              ap=[v0.ap[0], [192, 3], [128, 2], [1, 64]])
                nc.vector.tensor_copy(
                    out=dst, in_=ps.rearrange("p (a r j) -> p a r j", r=2, j=64))

    def vaug(kt, g, rev):
        # lhsT [128, 128]: [V_2g | ones] (rev=False, O rows 0:64) or
        # [ones | V_2g+1] (rev=True, O rows 64:128)
        if not rev:
            return vb[:, kt, 192 * g:192 * g + 128]
        return vb[:, kt, 192 * g + 64:192 * g + 192]

    mbf_v = mbf.rearrange("(a s) (t p) -> a t s p", s=512, p=128)

    def attn_quads(qt, g, mt, poA, poB, quads):
        for quad in quads:
            stA = stp.tile([128, 4, 512], bf16, tag="st", name=f"stA{qt}_{g}_{quad}")
            stB = stp.tile([128, 4, 512], bf16, tag="st", name=f"stB{qt}_{g}_{quad}")
            for par in range(2):
                peA = pep.tile([128, 2, 512], f32, tag="pe", name=f"peA{qt}_{g}_{quad}_{par}")
                peB = pep.tile([128, 2, 512], f32, tag="pe", name=f"peB{qt}_{g}_{quad}_{par}")
                for kk in range(2):
                    kt = quad * 4 + par * 2 + kk
                    nc.tensor.matmul(
                        out=peA[:, kk, :],
                        lhsT=ktp[0:64, g, kt * 128:(kt + 1) * 128],
                        rhs=qtp[0:64, g, qt * 512:(qt + 1) * 512],
                        start=True, stop=True)
                    nc.tensor.matmul(
                        out=peB[:, kk, :],
                        lhsT=ktp[64:128, g, kt * 128:(kt + 1) * 128],
                        rhs=qtp[64:128, g, qt * 512:(qt + 1) * 512],
                        start=True, stop=True)
                nc.scalar.activation(
                    out=stA[:, 2 * par:2 * par + 2, :], in_=peA,
                    func=Exp, scale=SCALE)
                nc.scalar.activation(
                    out=stB[:, 2 * par:2 * par + 2, :], in_=peB,
                    func=Exp, scale=SCALE)
            msl = mt[:, quad * 4:quad * 4 + 4, :]
            nc.vector.tensor_mul(out=stA, in0=stA, in1=msl)
            if debug_taps and qt == 0 and g == 0 and quad == 0:
                nc.gpsimd.dma_start(out=dbg["st"], in_=stA)
            nc.vector.tensor_mul(out=stB, in0=stB, in1=msl)
            for kk4 in range(4):
                kt = quad * 4 + kk4
                nc.tensor.matmul(
                    out=poA, lhsT=vaug(kt, g, False), rhs=stA[:, kk4, :],
                    start=(kt == 0), stop=(kt == KT - 1))
                nc.tensor.matmul(
                    out=poB, lhsT=vaug(kt, g, True), rhs=stB[:, kk4, :],
                    start=(kt == 0), stop=(kt == KT - 1))
    def attn_epilogue(qt, g, mt, poA, poB):
        if debug_taps and qt == 0 and g == 0:
            po_sb = osbp.tile([128, 512], f32, tag="dbgpo")
            nc.vector.tensor_copy(out=po_sb, in_=poA)
            nc.sync.dma_start(out=dbg["po"], in_=po_sb)
            nc.gpsimd.dma_start(out=dbg["mt"], in_=mt)
        # epilogue: divide O rows by Z rows, write attnT.
        # reciprocal_approx_fast (custom DVE op) only works at
        # partition base 0, so move Z there first for head A.
        rzA = rzp.tile([128, 2, 512], f32, tag="rz", name=f"rzA{qt}_{g}")
        nc.vector.tensor_copy(out=rzA[64:128, 0, :], in_=poA[64:128, :])
        nc.gpsimd.dma_start(out=rzA[0:64, 0, :], in_=rzA[64:128, 0, :])
        nc.vector.reciprocal_approx_fast(out=rzA[0:64, 1, :],
                                         in_=rzA[0:64, 0, :])
        nc.vector.tensor_mul(
            out=attnT[0:64, g, qt * 512:(qt + 1) * 512],
            in0=poA[0:64, :], in1=rzA[0:64, 1, :])
        if debug_taps and qt == 0 and g == 0:
            nc.sync.dma_start(out=dbg["rza"], in_=rzA[:, 1, :])
        rzB = rzp.tile([128, 2, 512], f32, tag="rz", name=f"rzB{qt}_{g}")
        nc.vector.reciprocal_approx_fast(out=rzB[0:64, 0, :],
                                         in_=poB[0:64, :])
        nc.gpsimd.dma_start(out=rzB[64:128, 0, :], in_=rzB[0:64, 0, :])
        nc.vector.tensor_mul(
            out=attnT[64:128, g, qt * 512:(qt + 1) * 512],
            in0=poB[64:128, :], in1=rzB[64:128, 0, :])

    def attn_head_pair(qt, g, mt):
        poA = pop.tile([128, 512], f32, tag="po", name=f"poA{qt}_{g}")
        poB = pop.tile([128, 512], f32, tag="po", name=f"poB{qt}_{g}")
        attn_quads(qt, g, mt, poA, poB, range(4))
        attn_epilogue(qt, g, mt, poA, poB)

    def fc_quarter(qt, q4):
        q8 = qt * 4 + q4
        osb = osbp.tile([128, E], f32, tag="osb", name=f"osb{q8}")
        for eh in range(2):
            pf_w = pep.tile([128, 2, 512], f32, tag="pe", name="psf")
            pf = pf_w[:, 0, 0:384]
            for fc in range(EC):
                nc.tensor.matmul(
                    out=pf, lhsT=attnT[:, fc, q8 * 128:(q8 + 1) * 128],
                    rhs=wfc_sb[:, fc, eh * 384:(eh + 1) * 384],
                    start=(fc == 0), stop=(fc == EC - 1))
            nc.scalar.activation(out=osb[:, eh * 384:(eh + 1) * 384], in_=pf,
                                 func=Copy)
        nc.sync.dma_start(out=Ox[q8 * 128:(q8 + 1) * 128, :], in_=osb)

    # ---- schedule (emission order must follow dataflow: Tile cannot
    # express a read waiting on a later-emitted write). g=0 of qt=0 is
    # split so its first-half exp/AV work starts right after the half-0
    # projections, overlapping the half-1 projections ----
    k_proj_half(0)
    mt0 = mtp.tile([128, KT, 512], bf16, tag="mt", name="mt0")
    for kt in range(KT):
        nc.sync.dma_start(out=mt0[:, kt, :], in_=mbf_v[0, kt], transpose=True)
    v_proj_half(0)
    poA0 = pop.tile([128, 512], f32, tag="po", name="poA0_0")
    poB0 = pop.tile([128, 512], f32, tag="po", name="poB0_0")
    attn_quads(0, 0, mt0, poA0, poB0, range(2))
    k_proj_half(1)
    v_proj_half(1)
    attn_quads(0, 0, mt0, poA0, poB0, range(2, 4))
    attn_epilogue(0, 0, mt0, poA0, poB0)
    for g in range(1, G):
        attn_head_pair(0, g, mt0)
    for q4 in range(4):
        fc_quarter(0, q4)

    mt1 = mtp.tile([128, KT, 512], bf16, tag="mt", name="mt1")
    for kt in range(KT):
        nc.sync.dma_start(out=mt1[:, kt, :], in_=mbf_v[1, kt], transpose=True)
    for g in range(G):
        attn_head_pair(1, g, mt1)
    for q4 in range(4):
        fc_quarter(1, q4)

    if debug_taps:
        nc.gpsimd.dma_start(out=dbg["qtp"], in_=qtp)
        nc.gpsimd.dma_start(out=dbg["ktp"], in_=ktp)
        nc.gpsimd.dma_start(out=dbg["vb"], in_=vb)
        nc.gpsimd.dma_start(out=dbg["attnT"], in_=attnT)


def _get_nc():
    if "nc" not in _CACHE:
        _CACHE["nc"] = _build()
    return _CACHE["nc"]


def kernel(Q, K, V, mask, Wq, Wk, Wv, Wfc, **_):
    from concourse.bass_utils import run_bass_kernel_spmd

    Q = np.asarray(Q, dtype=np.float32)
    K = np.asarray(K, dtype=np.float32)
    V = np.asarray(V, dtype=np.float32)
    mask = np.asarray(mask, dtype=np.int32)
    Wq = np.ascontiguousarray(np.asarray(Wq, dtype=np.float32))
    Wk = np.ascontiguousarray(np.asarray(Wk, dtype=np.float32))
    Wv = np.ascontiguousarray(np.asarray(Wv, dtype=np.float32))
    Wfc = np.ascontiguousarray(np.asarray(Wfc, dtype=np.float32))

    in_maps = []
    for c in range(N_CORES):
        b, qh = c // 2, c % 2
        in_maps.append({
            "Q": np.ascontiguousarray(Q[b, qh * QH:(qh + 1) * QH]),
            "K": np.ascontiguousarray(K[b]),
            "V": np.ascontiguousarray(V[b]),
            "mask": np.ascontiguousarray(mask[b, 0, qh * QH:(qh + 1) * QH]),
            "Wq": Wq, "Wk": Wk, "Wv": Wv, "Wfc": Wfc,
        })

    nc = _get_nc()
    res = run_bass_kernel_spmd(nc, in_maps, core_ids=list(range(N_CORES)))
    out = np.empty((B, S, E), dtype=np.float32)
    for c in range(N_CORES):
        b, qh = c // 2, c % 2
        out[b, qh * QH:(qh + 1) * QH] = res.results[c]["out"]
    return out
